# revision 29
# baseline (speedup 1.0000x reference)
"""BiDAF attention kernel for trn2 (8 NeuronCores, pure data parallel).

Final version: mixed-precision, descriptor-minimized DMA, software-
pipelined PE stream.

- The S = q W p chain needs >=14 mantissa bits (softmax logits have std
  ~256; single bf16's ~1.5 absolute logit error flips argmaxes and
  breaks both softmaxes — measured 0.55 rel err; a hardware probe showed
  float32r carries only ~9 bits, also unusable).  The entire chain runs
  as 3-term bf16 hi/lo products with fp32 PSUM accumulation: w and q^T
  ship as host-split bf16 pairs and Aq = Wh*Qh + Wh*Ql + Wl*Qh; the Aq
  result re-splits into a bf16 pair on PSUM exit; p^T ships as a bf16
  pair and S^T = Ph*Ah + Ph*Al + Pl*Ah.  Effective precision ~2^-16 per
  stage at the 1 cyc/row bf16 PE rate — hardware fp32 matmuls lower to
  2-pass LDW(LOW)+MM(LOW_HIGH) sequences costing ~2x more.  Everything
  after the exp is plain bf16 and the output blocks are written bf16
  (host upcasts).  End-to-end rel err 7.6e-3 (gate: 2e-2).
- All DRAM tensors are host-side pre-permuted to match the SBUF tile
  layouts exactly, so every DMA moves per-partition-contiguous 8-16KB
  runs (~100-128 descriptors/transfer).  HWDGE descriptor generation
  costs ~2ns/descriptor of sequencer time, and sub-2KB descriptors also
  cap HBM throughput, so this matters more than instruction count.
- DMA queue discipline: Wt and the first q^T slice go FIRST on the SP
  queue (the first Aq matmuls block on them, and ACT-queue packets get
  starved behind the SP queue's big p streams on the shared DMA
  engines); then the prefetch-critical p loads; later q slices and all
  stores issue from the ACT queue.  The last group's output is stored
  in per-batch quarters and its P*Ht runs on DVE (3.5x faster than
  GPSIMD) to shorten the kernel tail.
- Per-batch software pipelining: batch t's fp32 S^T matmuls are emitted
  one iteration ahead of its dependent PE ops (transposes, U, h), and
  the Ht broadcast (PE->DVE->ACT->PE chain) lags two iterations, so the
  in-order PE FIFO never head-of-line blocks on cross-engine round
  trips.  A short run of dummy matmuls bridges the initial DMA fill to
  keep the HAM clock-gate warm.
- The C2Q normalization is fused into the P*U elementwise op via
  scalar_tensor_tensor ((psU * rcp) * Pn, one DVE op per chunk); P*Ht
  runs on GPSIMD.  The Q2C stabilizer (per-batch global max) is computed
  per batch with a tiny transpose + ones-matmul broadcast, removing the
  4-batch barrier of the reference grouping.
- The P block of G is the verbatim (bf16-rounded) input and the Ht block
  is 400 identical rows; both are materialized during host-side unshard
  (the device computes and writes h once per batch) instead of streaming
  ~6.6MB/core of redundant copies through HBM.
"""

from contextlib import ExitStack

import numpy as np
import ml_dtypes

import concourse.bass as bass
import concourse.mybir as mybir
import concourse.tile as tile
from concourse.bass_utils import run_bass_kernel_spmd
from concourse.masks import make_identity

F32 = mybir.dt.float32
BF16 = mybir.dt.bfloat16
AX = mybir.AxisListType
ALU = mybir.AluOpType
ACTF = mybir.ActivationFunctionType

B, LP, LQ, H = 128, 400, 100, 256
NCORES = 8
BP = B // NCORES   # batches per core
NG = BP // 4       # 4-batch groups per core
R = 100            # LP chunk rows (4 uniform chunks)


def build_nc():
    nc = bass.Bass("TRN2", target_bir_lowering=False, debug=False)

    # host-permuted layouts (match SBUF tiles exactly)
    pnp = nc.dram_tensor("pnp", [NG, R, 4, 4, H], BF16, kind="ExternalInput")
    ptp = nc.dram_tensor("ptp", [NG, 128, 4, 2, 2, LP], BF16,
                         kind="ExternalInput")
    qtp = nc.dram_tensor("qtp", [128, 2, 2, BP * LQ], BF16,
                         kind="ExternalInput")
    qnp = nc.dram_tensor("qnp", [LQ, BP, H], BF16, kind="ExternalInput")
    wtp = nc.dram_tensor("wtp", [128, 2, 2, H], BF16, kind="ExternalInput")
    gHt = nc.dram_tensor("gHt", [NG, 4, H], BF16, kind="ExternalOutput")
    gG2 = nc.dram_tensor("gG2", [NG, R, 4, 4, 2 * H], BF16,
                         kind="ExternalOutput")

    with tile.TileContext(nc) as tc, ExitStack() as ctx:
        cpool = ctx.enter_context(tc.tile_pool(name="consts", bufs=1))
        wp = ctx.enter_context(tc.tile_pool(name="work", bufs=2))
        pst = ctx.enter_context(tc.tile_pool(name="pst", bufs=2, space="PSUM"))
        pam = ctx.enter_context(tc.tile_pool(name="pam", bufs=2, space="PSUM"))
        ppu = ctx.enter_context(tc.tile_pool(name="ppu", bufs=2, space="PSUM"))
        ppt = ctx.enter_context(tc.tile_pool(name="ppt", bufs=2, space="PSUM"))

        # ---- constants ----
        identF = cpool.tile([128, 128], F32)
        make_identity(nc, identF[:])
        identB = cpool.tile([128, 128], BF16)
        make_identity(nc, identB[:])
        ones_rF = cpool.tile([1, 128], F32)
        nc.vector.memset(ones_rF[:], 1.0)
        onesCb = cpool.tile([128, 1], BF16)
        nc.vector.memset(onesCb[:], 1.0)
        ones_rB = cpool.tile([1, 128], BF16)
        nc.vector.memset(ones_rB[:], 1.0)

        Wt = cpool.tile([128, 2, 2, H], BF16)
        nc.sync.dma_start(Wt[:], wtp[:])
        QtA = cpool.tile([128, 2, 2, BP * LQ], BF16)
        AhA = cpool.tile([128, 2, BP * LQ], BF16)
        AlA = cpool.tile([128, 2, BP * LQ], BF16)
        QnA = cpool.tile([LQ, BP, H], BF16)

        def load_qt(gi, eng=None):
            (eng or nc.scalar).dma_start(
                QtA[:, :, :, gi * 400:(gi + 1) * 400],
                qtp[:, :, :, gi * 400:(gi + 1) * 400])

        def compute_aq(gi):
            # Aq[d, (b l)] = sum_h w[h, d] q[b, l, h]
            # 3-term bf16 hi/lo product (Wh*Qh + Wh*Ql + Wl*Qh), f32 accum
            sl = slice(gi * 400, (gi + 1) * 400)
            for ms in range(2):
                psAq = pst.tile([128, 400], F32, tag="st")
                for kc in range(2):
                    Wh = Wt[:, kc, 0, ms * 128:(ms + 1) * 128]
                    Wl = Wt[:, kc, 1, ms * 128:(ms + 1) * 128]
                    nc.tensor.matmul(psAq[:], Wh, QtA[:, kc, 0, sl],
                                     start=(kc == 0), stop=False)
                    nc.tensor.matmul(psAq[:], Wh, QtA[:, kc, 1, sl],
                                     start=False, stop=False)
                    nc.tensor.matmul(psAq[:], Wl, QtA[:, kc, 0, sl],
                                     start=False, stop=(kc == 1))
                nc.scalar.copy(AhA[:, ms, sl], psAq[:])
                nc.vector.tensor_tensor(AlA[:, ms, sl], psAq[:],
                                        AhA[:, ms, sl], op=ALU.subtract)

        # keep the PE busy (HAM-warm) while the first inputs stream in
        # (must be real matmuls: transpose-mode does not count as PE-busy
        # for the HAM clock-gate)
        psW = pst.tile([128, 128], F32, tag="st")
        for _ in range(24):
            nc.tensor.matmul(psW[:], identB[:], identB[:],
                             start=True, stop=True)

        load_qt(0, nc.sync)
        PtGs, PnGs = {}, {}

        def load_group(gi):
            PtG_t = wp.tile([128, 4, 2, 2, LP], BF16, tag="PtG", bufs=3,
                            name=f"PtG{gi}")
            nc.sync.dma_start(PtG_t[:], ptp[gi])
            PtGs[gi] = PtG_t
            PnG_t = wp.tile([R, 4, 4, H], BF16, tag="PnG", bufs=3,
                            name=f"PnG{gi}")
            nc.sync.dma_start(PnG_t[:], pnp[gi])
            PnGs[gi] = PnG_t

        load_group(0)
        compute_aq(0)
        nc.sync.dma_start(QnA[:], qnp[:])

        # ------------- software-pipelined main loop -------------
        # Batch t's S^T matmuls are emitted one iteration ahead of its
        # dependent PE ops (transposes, U, h, Ht) so the PE FIFO never
        # head-of-line blocks on DVE/ACT round-trips; the Ht broadcast
        # lags one further iteration (it sits behind a PE->DVE->ACT->PE
        # chain).
        NB = NG * 4
        st_tiles = {}
        state = {}
        group_of = lambda t: t // 4

        def emit_st(t):
            gi, j = divmod(t, 4)
            if j == 0 and gi > 0:
                compute_aq(gi)
            if j == 0 and gi + 1 < NG:
                load_group(gi + 1)
                load_qt(gi + 1)
            if j == 0:
                gtg = wp.tile([R, 4, 4, 2 * H], BF16, tag="GtG", bufs=3,
                              name=f"GtG{gi}")
                htg = wp.tile([R, 4, H], BF16, tag="HtG", bufs=3,
                              name=f"HtG{gi}")
                state[gi] = (gtg, htg)
            psSt = pst.tile([R, 4, LQ], F32, tag="st", name=f"psSt{t}")
            bq = t * LQ
            for i in range(4):
                for kc in range(2):
                    Ph = PtGs[gi][:, j, kc, 0, i * R:(i + 1) * R]
                    Pl = PtGs[gi][:, j, kc, 1, i * R:(i + 1) * R]
                    nc.tensor.matmul(psSt[:, i, :], Ph,
                                     AhA[:, kc, bq:bq + LQ],
                                     start=(kc == 0), stop=False)
                    nc.tensor.matmul(psSt[:, i, :], Ph,
                                     AlA[:, kc, bq:bq + LQ],
                                     start=False, stop=False)
                    nc.tensor.matmul(psSt[:, i, :], Pl,
                                     AhA[:, kc, bq:bq + LQ],
                                     start=False, stop=(kc == 1))
            st_tiles[t] = psSt

        def emit_ht_tail(t):
            # psHt(t) + HtG copy + PHt + (group stores when t ends a group)
            gi, j = divmod(t, 4)
            GtG, HtG = state[gi]
            hrow = pipe[t]["hrow"]
            psHt = ppt.tile([128, 260], F32, tag="tiny", name=f"psHt{t}")
            nc.tensor.matmul(psHt[0:R, 0:H], ones_rB[0:1, 0:R],
                             hrow[:], start=True, stop=True)
            nc.scalar.copy(HtG[:, j, :], psHt[0:R, 0:H])
            if t >= NB - 2:
                # kernel tail: DVE is ~3.5x faster per element than GPSIMD
                nc.vector.tensor_tensor(
                    GtG[:, j, :, H:2 * H], PnGs[gi][:, j, :, :],
                    HtG[:, j:j + 1, :].broadcast_to((R, 4, H)), op=ALU.mult)
            else:
                nc.gpsimd.tensor_tensor(
                    GtG[:, j, :, H:2 * H], PnGs[gi][:, j, :, :],
                    HtG[:, j:j + 1, :].broadcast_to((R, 4, H)), op=ALU.mult)
            last_group = gi == NG - 1
            if last_group:
                # fire each batch's slice as soon as its P*Ht lands
                nc.scalar.dma_start(gG2[gi][:, j:j + 1, :, :],
                                    GtG[:, j:j + 1, :, :])
            elif j == 1:
                nc.scalar.dma_start(gG2[gi][:, 0:2, :, :], GtG[:, 0:2, :, :])
            if j == 3:
                nc.scalar.dma_start(gHt[gi], HtG[0:1, :, :])
                if not last_group:
                    nc.scalar.dma_start(gG2[gi][:, 2:4, :, :],
                                        GtG[:, 2:4, :, :])
                PtGs.pop(gi)
                PnGs.pop(gi)

        pipe = {}
        emit_st(0)
        for t in range(NB):
            gi, j = divmod(t, 4)
            if t - 2 in pipe:
                emit_ht_tail(t - 2)
                del pipe[t - 2]
            if t + 1 < NB:
                emit_st(t + 1)
            psSt = st_tiles.pop(t)
            GtG, HtG = state[gi]
            PnG = PnGs[gi]

            NM = wp.tile([R, 4], F32, tag="NM", bufs=4, name=f"NM{t}")
            nc.vector.tensor_reduce(NM[:], psSt[:], axis=AX.X,
                                    op=ALU.max, negate=True)
            E = wp.tile([R, 4, LQ], BF16, tag="E", bufs=3, name=f"E{t}")
            for i in range(4):
                nc.scalar.activation(
                    E[:, i, :], psSt[:, i, :], ACTF.Exp,
                    bias=NM[:, i:i + 1],
                )
            # per-batch Q2C stabilizer: gmax_b = -min over all chunks
            NMN = wp.tile([R, 1], F32, tag="NMN", name=f"NMN{t}")
            nc.vector.tensor_reduce(NMN[:], NM[:], axis=AX.X, op=ALU.min)
            psT = ppt.tile([128, 260], F32, tag="tiny", name=f"psT{t}")
            nc.tensor.transpose(psT[0:1, 0:R], NMN[:], identF[0:R, 0:R])
            gneg = wp.tile([1, 1], F32, tag="gneg", name=f"gneg{t}")
            nc.vector.tensor_reduce(gneg[:], psT[0:1, 0:R], axis=AX.X,
                                    op=ALU.min)
            psB = ppt.tile([128, 260], F32, tag="tiny", name=f"psB{t}")
            nc.tensor.matmul(psB[0:R, 0:1], ones_rF[0:1, 0:R], gneg[:],
                             start=True, stop=True)
            nbv = wp.tile([R, 1], F32, tag="nbv", name=f"nbv{t}")
            nc.vector.tensor_copy(nbv[:], psB[0:R, 0:1])

            # a = E^T (unnormalized); U^T = a_chunk @ Qn
            psAm = pam.tile([R, 4, LQ], BF16, tag="am", name=f"psAm{t}")
            for i in range(4):
                nc.tensor.transpose(
                    psAm[0:LQ, i, 0:R], E[:, i, :], identB[0:R, 0:R])
            Am = wp.tile([LQ, 4, R], BF16, tag="Am", bufs=3, name=f"Am{t}")
            nc.scalar.copy(Am[:], psAm[0:LQ, :, :])
            RS = wp.tile([R, 4], F32, tag="RS", name=f"RS{t}")
            nc.vector.tensor_reduce(RS[:], E[:], axis=AX.X, op=ALU.add)
            RCP = wp.tile([R, 4], F32, tag="RCP", name=f"RCP{t}")
            nc.vector.reciprocal(RCP[:], RS[:])
            psUs = []
            for half in range(2):
                psU = ppu.tile([R, 2, H], F32, tag="u", name=f"psU{t}_{half}")
                psUs.append(psU)
                for ih in range(2):
                    i = half * 2 + ih
                    nc.tensor.matmul(
                        psU[:, ih, :], Am[:, i, :], QnA[:, t, :],
                        start=True, stop=True,
                    )
            for half in range(2):
                for ih in range(2):
                    i = half * 2 + ih
                    nc.vector.scalar_tensor_tensor(
                        GtG[:, j, i, 0:H],
                        psUs[half][:, ih, :], RCP[:, i:i + 1],
                        PnG[:, j, i, :],
                        op0=ALU.mult, op1=ALU.mult,
                    )
            # Q2C h
            EQ = wp.tile([R, 4], BF16, tag="EQ", name=f"EQ{t}")
            nc.scalar.activation(EQ[:], NM[:], ACTF.Exp,
                                 bias=nbv[:, 0:1], scale=-1.0)
            psHr = ppt.tile([128, 260], F32, tag="tiny", name=f"psHr{t}")
            for i in range(4):
                nc.tensor.matmul(
                    psHr[0:1, 0:H],
                    EQ[:, i:i + 1],
                    PnG[:, j, i, :],
                    start=(i == 0), stop=(i == 3),
                )
            nc.tensor.matmul(psHr[0:1, H:H + 4], onesCb[0:R, :], EQ[:],
                             start=True, stop=True)
            smv = wp.tile([1, 1], F32, tag="smv", name=f"smv{t}")
            nc.vector.tensor_reduce(smv[:], psHr[0:1, H:H + 4],
                                    axis=AX.X, op=ALU.add)
            rq = wp.tile([1, 1], F32, tag="rq", name=f"rq{t}")
            nc.vector.reciprocal(rq[:], smv[:])
            hrow = wp.tile([1, H], BF16, tag="hrow", name=f"hrow{t}")
            nc.scalar.mul(hrow[:], psHr[0:1, 0:H], rq[:])
            pipe[t] = {"hrow": hrow}
        emit_ht_tail(NB - 2)
        emit_ht_tail(NB - 1)

    return nc


def legalize_waits(nc):
    """Split multi-wait instructions into single-wait NoOps + instruction.

    The TPB ISA has exactly one (wait, update) EVENTS slot per 64B
    instruction; this walrus build refuses instructions with more than one
    sync wait ("Too many sync wait commands").  Tile's scheduler emits
    vector-clock waits freely, so legalize here: excess waits move onto
    engine-queue NoOps placed immediately before the instruction.
    """
    counter = 0
    for f in nc.m.functions:
        for blk in f.blocks:
            new = []
            for inst in blk.instructions:
                si = getattr(inst, "sync_info", None)
                if si is not None and len(si.on_wait) > 1:
                    waits = list(si.on_wait)
                    assert len(si.on_update) <= 1, inst
                    for wt in waits[:-1]:
                        counter += 1
                        new.append(mybir.InstNoOp(
                            name=f"I-waitnop-{counter}",
                            engine=inst.engine,
                            sync_info=mybir.SyncInfo(on_wait=[wt],
                                                     on_update=[]),
                        ))
                    inst.sync_info = mybir.SyncInfo(
                        on_wait=[waits[-1]], on_update=list(si.on_update))
                new.append(inst)
            blk.instructions = new
    return nc


def _make_in_maps(p, q, w):
    p = np.ascontiguousarray(p, dtype=np.float32)
    q = np.ascontiguousarray(q, dtype=np.float32)
    w = np.ascontiguousarray(w, dtype=np.float32)
    bf = ml_dtypes.bfloat16
    in_maps = []
    for c in range(NCORES):
        sl = slice(c * BP, (c + 1) * BP)
        ps = p[sl]
        qs = q[sl]
        # pnp[gi, r, jb, i, :] = p[b0+jb, i*100+r, :]
        pnp = np.ascontiguousarray(
            ps.reshape(NG, 4, 4, R, H).transpose(0, 3, 1, 2, 4).astype(bf))
        # ptp[gi, d, jb, kc, hl, l] = hi/lo bf16 split of p[b0+jb, l, kc*128+d]
        pt_full = ps.transpose(0, 2, 1)
        pt_hi = pt_full.astype(bf)
        pt_lo = (pt_full - pt_hi.astype(np.float32)).astype(bf)
        ptp = np.ascontiguousarray(np.stack(
            [x.reshape(NG, 4, 2, 128, LP).transpose(0, 3, 1, 2, 4)
             for x in (pt_hi, pt_lo)], axis=4))
        # qtp[d, kc, hl, b*100+l] = hi/lo bf16 split of q[b, l, kc*128+d]
        qt_f = qs.transpose(2, 0, 1).reshape(2, 128, BP * LQ).transpose(1, 0, 2)
        qt_h = qt_f.astype(bf)
        qt_l = (qt_f - qt_h.astype(np.float32)).astype(bf)
        qtp = np.ascontiguousarray(np.stack([qt_h, qt_l], axis=2))
        qnp = np.ascontiguousarray(qs.transpose(1, 0, 2).astype(bf))
        wt_f = w.reshape(2, 128, H).transpose(1, 0, 2)
        wt_h = wt_f.astype(bf)
        wt_l = (wt_f - wt_h.astype(np.float32)).astype(bf)
        wtp = np.ascontiguousarray(np.stack([wt_h, wt_l], axis=2))
        in_maps.append({"pnp": pnp, "ptp": ptp, "qtp": qtp, "qnp": qnp,
                        "wtp": wtp})
    return in_maps


def _assemble(res_c, pnp):
    """Rebuild (BP, LP, 4H) float32 from the permuted device outputs.

    The P block of G is the verbatim (bf16-rounded) input p; it is placed
    during unshard rather than round-tripped through device HBM.
    """
    out = np.empty((BP, LP, 4 * H), np.float32)
    gHt = np.asarray(res_c["gHt"])    # (NG, 4, H)
    gG2 = np.asarray(res_c["gG2"])    # (NG, R, 4, 4, 2H)
    out[:, :, 0:H] = (
        pnp.transpose(0, 2, 3, 1, 4).reshape(BP, LP, H).astype(np.float32))
    out[:, :, H:2 * H] = np.broadcast_to(
        gHt.reshape(BP, 1, H).astype(np.float32), (BP, LP, H))
    out[:, :, 2 * H:4 * H] = (
        gG2.transpose(0, 2, 3, 1, 4).reshape(BP, LP, 2 * H)
        .astype(np.float32))
    return out


def run(p, q, w, trace=False):
    nc = legalize_waits(build_nc())
    maps = _make_in_maps(p, q, w)
    res = run_bass_kernel_spmd(nc, maps, list(range(NCORES)), trace=trace)
    out = np.concatenate(
        [_assemble(res.results[c], maps[c]["pnp"]) for c in range(NCORES)],
        axis=0)
    return out, res


def kernel(p, q, w):
    out, _ = run(p, q, w, trace=False)
    return out
